# revision 1
# baseline (speedup 1.0000x reference)
"""Longformer attention TP-sharded Bass kernel for 8 NeuronCores.

Sharding: tensor-parallel over heads. Core d owns heads 2d, 2d+1:
  - Wq/Wk/Wv rows [128d:128(d+1)]  (nn.Linear: q = x @ Wq.T)
  - Wo columns [128d:128(d+1)]
  Each core computes its heads' sparse (windowed+global) attention and a
  full-size out-proj partial; host sums the 8 partials (the "all-reduce").

Device layout (all bf16 compute, fp32 PSUM accumulate):
  xT  [1024h, 4096s]  - x transposed (host prep) so hidden is contraction dim
  qT/kT [128o, 4096s] - head dims on partitions (head A: 0-63, head B: 64-127)
  v   [128s, 32kb, 130] - natural layout per key block, with a ones column per
                          head so the PV matmul also produces the softmax
                          denominator (col 64 / col 129).
  scores are computed transposed [k, q]: softmax sum over k comes out of the
  PE via the ones column; masks are multiplicative 0/1 on exp(scores) (safe:
  scores are O(1) here, no max-subtraction needed).
"""

import os
import numpy as np
import ml_dtypes

S = 4096
HIDDEN = 1024
N_CORES = 8
OC = 128          # out-proj contraction dims (head dims) per core = 2 heads x 64
NQB = S // 128    # 32 query/key blocks
BF16 = ml_dtypes.bfloat16

_CACHE = {}
LAST_RESULTS = None


def _masks_np():
    """Per-group-class multiplicative masks, pre-concatenated along the key
    blocks of one PSUM group, scoresT [k(partition), q(free)] layout.
    Layout [5, 128, 512]:
      0: mid  [row0 | lo | ones | up]   (qb in 2..30)
      1: q1   [lo0  | ones | up | pad]  (qb == 1, width 384)
      2: q31  [row0 | lo | ones | pad]  (qb == 31, width 384)
      3: q0a  [ones | up0 | col0 | col0] (qb == 0, first group)
      4: q0b  [col0 x4]                  (qb == 0, groups 1..7)
    """
    p = np.arange(128)[:, None]   # key index within block
    f = np.arange(128)[None, :]   # query index within block
    ones = np.ones((128, 128), bool)
    m_lo = (f <= p)
    m_lo0 = m_lo | (p == 0)
    m_up = (f >= p)
    m_up0 = m_up | (f == 0)
    m_row0 = np.broadcast_to(p == 0, (128, 128))
    m_col0 = np.broadcast_to(f == 0, (128, 128))
    out = np.zeros((5, 128, 512), bool)
    out[0] = np.concatenate([m_row0, m_lo, ones, m_up], 1)
    out[1, :, :384] = np.concatenate([m_lo0, ones, m_up], 1)
    out[2, :, :384] = np.concatenate([m_row0, m_lo, ones], 1)
    out[3] = np.concatenate([ones, m_up0, m_col0, m_col0], 1)
    out[4] = np.concatenate([m_col0] * 4, 1)
    return out.astype(BF16)


def _mask_idx_for(qb, g0):
    """Mask slot for the group starting at block-list offset g0, or None."""
    if qb == 0:
        return 3 if g0 == 0 else 4
    if qb == 1:
        return 1
    if qb == NQB - 1:
        return 2
    return 0


def _kbs_for(qb):
    """[(key_block, mask_idx or None)] for query block qb."""
    if qb == 0:
        return [(0, None), (1, 3)] + [(kb, 5) for kb in range(2, NQB)]
    if qb == 1:
        return [(0, 1), (1, None), (2, 2)]
    if qb == NQB - 1:
        return [(0, 4), (qb - 1, 0), (qb, None)]
    return [(0, 4), (qb - 1, 0), (qb, None), (qb + 1, 2)]


def _build():
    import concourse.bass as bass
    import concourse.mybir as mybir
    import concourse.tile as tile
    from concourse import bacc

    f32 = mybir.dt.float32
    bf16 = mybir.dt.bfloat16
    Exp = mybir.ActivationFunctionType.Exp

    nc = bacc.Bacc("TRN2", target_bir_lowering=False, debug=False,
                   num_devices=N_CORES)

    xt_d = nc.dram_tensor("xt", [HIDDEN, S], bf16, kind="ExternalInput").ap()
    wq_d = nc.dram_tensor("wqt", [HIDDEN, OC], bf16, kind="ExternalInput").ap()
    wk_d = nc.dram_tensor("wkt", [HIDDEN, OC], bf16, kind="ExternalInput").ap()
    wv_d = nc.dram_tensor("wvt", [HIDDEN, OC], bf16, kind="ExternalInput").ap()
    wo_d = nc.dram_tensor("wot", [OC, HIDDEN], bf16, kind="ExternalInput").ap()
    out_d = nc.dram_tensor("partial", [S, HIDDEN], bf16,
                           kind="ExternalOutput").ap()
    mask_d = nc.inline_tensor(_masks_np(), name="masks").ap()
    id_d = nc.inline_tensor(np.eye(128, dtype=BF16), name="ident").ap()

    with tile.TileContext(nc) as tc:
        import contextlib
        with contextlib.ExitStack() as ctx:
            big = ctx.enter_context(tc.tile_pool(name="big", bufs=1))
            tmp = ctx.enter_context(tc.tile_pool(name="tmp", bufs=3))
            psb = ctx.enter_context(tc.tile_pool(name="psb", bufs=3, space="PSUM"))
            pso = ctx.enter_context(tc.tile_pool(name="pso", bufs=2, space="PSUM"))
            pst = ctx.enter_context(tc.tile_pool(name="pst", bufs=2, space="PSUM"))

            # ---- resident tensors ----
            xt_sb = big.tile([128, 8, S], bf16)       # x.T, hidden chunks on dim1
            qt_sb = big.tile([128, S], bf16)          # q.T * 0.125
            kt_sb = big.tile([128, S], bf16)
            v_sb = big.tile([128, NQB, 130], bf16)    # [vA|1|vB|1] per key block
            outn_sb = big.tile([128, NQB, 128], bf16)  # attn out, natural [q, hd]
            outt_sb = big.tile([128, NQB, 128], bf16)  # transposed [hd, q]
            wq_sb = big.tile([128, 8, OC], bf16)
            wk_sb = big.tile([128, 8, OC], bf16)
            wv_sb = big.tile([128, 8, OC], bf16)
            wo_sb = big.tile([128, HIDDEN], bf16)
            mask_sb = big.tile([128, 5, 512], bf16)
            id_sb = big.tile([128, 128], bf16)

            # ---- constant / weight loads ----
            nc.sync.dma_start(wq_sb, wq_d.rearrange("(c p) o -> p c o", p=128))
            nc.sync.dma_start(wk_sb, wk_d.rearrange("(c p) o -> p c o", p=128))
            nc.sync.dma_start(wv_sb, wv_d.rearrange("(c p) o -> p c o", p=128))
            nc.sync.dma_start(wo_sb, wo_d)
            nc.sync.dma_start(mask_sb, mask_d.rearrange("m p f -> p m f"))
            nc.sync.dma_start(id_sb, id_d)
            nc.vector.memset(v_sb[:, :, 64], 1.0)
            nc.vector.memset(v_sb[:, :, 129], 1.0)

            xt_ap = xt_d.rearrange("(c p) s -> p c s", p=128)

            # ---- phase A: projections ----
            for sc in range(8):
                ssl = slice(sc * 512, (sc + 1) * 512)
                nc.sync.dma_start(xt_sb[:, :, ssl], xt_ap[:, :, ssl])

                psq = psb.tile([128, 512], f32, tag="ps512", name="psq")
                for hc in range(8):
                    nc.tensor.matmul(psq, wq_sb[:, hc, :], xt_sb[:, hc, ssl],
                                     start=(hc == 0), stop=(hc == 7))
                # fold the 1/sqrt(hd) = 0.125 softmax scale into q
                nc.vector.tensor_scalar_mul(qt_sb[:, ssl], psq, 0.125)

                psk = psb.tile([128, 512], f32, tag="ps512", name="psk")
                for hc in range(8):
                    nc.tensor.matmul(psk, wk_sb[:, hc, :], xt_sb[:, hc, ssl],
                                     start=(hc == 0), stop=(hc == 7))
                nc.vector.tensor_copy(kt_sb[:, ssl], psk)

                for b in range(4):
                    kb = sc * 4 + b
                    bsl = slice(sc * 512 + b * 128, sc * 512 + b * 128 + 128)
                    psv = psb.tile([128, 512], f32, tag="ps512", name="psv")
                    for hc in range(8):
                        nc.tensor.matmul(psv[:, :128], xt_sb[:, hc, bsl],
                                         wv_sb[:, hc, :],
                                         start=(hc == 0), stop=(hc == 7))
                    # single strided copy: [vA(64) -> col 0] and [vB -> col 65]
                    vdst = v_sb[:, kb, :].rearrange("p (h c) -> p h c", h=2)
                    nc.vector.tensor_copy(
                        vdst[:, :, 0:64],
                        psv[:, 0:128].rearrange("p (h c) -> p h c", h=2))

            # ---- phase B + C interleaved per query block ----
            for qb in range(NQB):
                qsl = slice(qb * 128, (qb + 1) * 128)
                for h in range(2):
                    bp = 64 * h
                    blocks = _kbs_for(qb)
                    nmm = len(blocks)
                    pso_t = pso.tile([128, 65], f32, tag="psO", name="pso_t")
                    mmi = 0
                    for g0 in range(0, nmm, 4):
                        grp = blocks[g0:g0 + 4]
                        gw = 128 * len(grp)
                        pss = psb.tile([128, 512], f32, tag="ps512", name="pss")
                        for j, (kb, mi) in enumerate(grp):
                            nc.tensor.matmul(
                                pss[:, j * 128:(j + 1) * 128],
                                kt_sb[bp:bp + 64, kb * 128:(kb + 1) * 128],
                                qt_sb[bp:bp + 64, qsl],
                                start=True, stop=True)
                        probs = tmp.tile([128, 512], bf16, tag="probs",
                                         name="probs")
                        nc.scalar.activation(probs[:, :gw], pss[:, :gw], Exp)
                        mig = _mask_idx_for(qb, g0)
                        nc.vector.tensor_mul(probs[:, :gw], probs[:, :gw],
                                             mask_sb[:, mig, :gw])
                        for j, (kb, mi) in enumerate(grp):
                            nc.tensor.matmul(
                                pso_t, probs[:, j * 128:(j + 1) * 128],
                                v_sb[:, kb, 65 * h:65 * h + 65],
                                start=(mmi == 0), stop=(mmi == nmm - 1),
                                skip_group_check=True)
                            mmi += 1
                    recip = tmp.tile([128, 1], f32, tag="recip", name="recip")
                    nc.vector.reciprocal(recip, pso_t[:, 64:65])
                    nc.vector.tensor_scalar_mul(
                        outn_sb[:, qb, 64 * h:64 * h + 64],
                        pso_t[:, 0:64], recip)

                # out-proj for this query block (overlaps later qbs' attention)
                pstr = pst.tile([128, 128], bf16, tag="psT", name="pstr")
                nc.tensor.transpose(pstr, outn_sb[:, qb, :], id_sb)
                nc.vector.tensor_copy(outt_sb[:, qb, :], pstr)
                stage = tmp.tile([128, HIDDEN], bf16, tag="stage", name="stage")
                for oc in range(2):
                    psp = psb.tile([128, 512], f32, tag="ps512", name="psp")
                    nc.tensor.matmul(psp, outt_sb[:, qb, :],
                                     wo_sb[:, oc * 512:(oc + 1) * 512],
                                     start=True, stop=True)
                    if oc == 0:
                        nc.vector.tensor_copy(
                            stage[:, oc * 512:(oc + 1) * 512], psp)
                    else:
                        nc.scalar.copy(stage[:, oc * 512:(oc + 1) * 512], psp)
                nc.sync.dma_start(out_d[qb * 128:(qb + 1) * 128, :], stage)

    nc.compile()
    return nc


def kernel(x, Wq, Wk, Wv, Wo):
    from concourse import bass_utils

    x = np.asarray(x)
    B = x.shape[0]
    xt = np.ascontiguousarray(np.asarray(x)[0].T.astype(BF16))
    in_maps = []
    for d in range(N_CORES):
        rs = slice(OC * d, OC * (d + 1))
        in_maps.append({
            "xt": xt,
            "wqt": np.ascontiguousarray(np.asarray(Wq)[rs, :].T.astype(BF16)),
            "wkt": np.ascontiguousarray(np.asarray(Wk)[rs, :].T.astype(BF16)),
            "wvt": np.ascontiguousarray(np.asarray(Wv)[rs, :].T.astype(BF16)),
            "wot": np.ascontiguousarray(np.asarray(Wo)[:, rs].T.astype(BF16)),
        })

    if "nc" not in _CACHE:
        _CACHE["nc"] = _build()
    nc = _CACHE["nc"]

    res = bass_utils.run_bass_kernel_spmd(
        nc, in_maps, core_ids=list(range(N_CORES)),
        trace=bool(os.environ.get("KERNEL_TRACE")))
    global LAST_RESULTS
    LAST_RESULTS = res

    out = np.zeros((S, HIDDEN), np.float64)
    for r in res.results:
        out += r["partial"].astype(np.float64)
    return out.reshape(B, S, HIDDEN).astype(np.float32)



# revision 16
# speedup vs baseline: 1.5707x; 1.5707x over previous
"""Longformer attention TP-sharded Bass kernel for 8 NeuronCores (v2).

Sharding: tensor-parallel over heads. Core d owns heads 2d, 2d+1:
  - Wq/Wk/Wv rows [128d:128(d+1)]  (nn.Linear: q = x @ Wq.T)
  - Wo columns [128d:128(d+1)]
  Each core computes its heads' sparse (windowed+global) attention and a
  full-size out-proj partial; host sums the 8 partials (the "all-reduce").

v2 layout (bf16 compute, fp32 PSUM):
  scoresT [k, q] per query block qb as ONE psum tile [128, W]:
    [A-lo | A-diag | A-up | B-lo | B-diag | B-up | key0-strips]
  where lo/diag/up are the 3 banded key blocks (384 cols/head), and the
  key0 "strips" are [1, 128] rows (key-0 scores for this qb's queries),
  head A at partition 0, head B at partition 32 (PE quadrant offsets).
  One exp over the whole tile; one gpsimd mask-multiply over the band
  part; PV via probs-stationary matmuls plus a rank-1 key0 term using a
  v[0]-broadcast tile so base partitions match.
  The global query row (q=0) accumulates via N=1 rider matmuls on the
  diag stationaries into a persistent [128, 64] psum tile (col = 2qb+h),
  one exp at the end, then 60 tiny PV matmuls; qb 0's normalize/out-proj
  is deferred to the end so the q0 row and denominator can be patched.
"""

import os
import numpy as np
import ml_dtypes

S = 4096
HIDDEN = 1024
N_CORES = 8
OC = 128          # out-proj contraction dims (head dims) per core = 2 heads x 64
NQB = S // 128    # 32 query/key blocks
BF16 = ml_dtypes.bfloat16

_CACHE = {}
LAST_RESULTS = None


def _masks_np():
    """Multiplicative band masks, 2-head-concatenated, scoresT [k, q] layout.
    [4, 128, 768]:
      0: mid  [lo | ones | up] x2
      1: qb1  [lo&(k>0) | ones | up] x2
      2: qb31 [lo | ones] x2 (512 used)
      3: qb0  [ones | up|(q==0)] x2 (512 used)
    """
    p = np.arange(128)[:, None]   # key index within block
    f = np.arange(128)[None, :]   # query index within block
    ones = np.ones((128, 128), bool)
    lo = (f <= p)
    up = (f >= p)
    up0 = up | (f == 0)
    lo_nok0 = lo & (p > 0)
    out = np.zeros((4, 128, 768), bool)
    out[0] = np.tile(np.concatenate([lo, ones, up], 1), (1, 2))
    out[1] = np.tile(np.concatenate([lo_nok0, ones, up], 1), (1, 2))
    out[2, :, :512] = np.tile(np.concatenate([lo, ones], 1), (1, 2))
    out[3, :, :512] = np.tile(np.concatenate([ones, up0], 1), (1, 2))
    return out.astype(BF16)


def _qb_plan(qb):
    """Return (kbs, width_per_head, mask_slot, strip_off or None).
    kbs: list of key blocks per head (score col = j*128 within the head
    section)."""
    if qb == 0:
        return [0, 1], 256, 3, None
    if qb == 1:
        return [0, 1, 2], 384, 1, 768
    if qb == NQB - 1:
        return [NQB - 2, NQB - 1], 256, 2, 512
    return [qb - 1, qb, qb + 1], 384, 0, 768


def _build():
    import concourse.bass as bass
    import concourse.mybir as mybir
    import concourse.tile as tile
    from concourse import bacc

    f32 = mybir.dt.float32
    bf16 = mybir.dt.bfloat16
    Exp = mybir.ActivationFunctionType.Exp

    nc = bacc.Bacc("TRN2", target_bir_lowering=False, debug=False,
                   num_devices=N_CORES)

    xt_d = nc.dram_tensor("xt", [HIDDEN, S], bf16, kind="ExternalInput").ap()
    wq_d = nc.dram_tensor("wqt", [HIDDEN, OC], bf16, kind="ExternalInput").ap()
    wk_d = nc.dram_tensor("wkt", [HIDDEN, OC], bf16, kind="ExternalInput").ap()
    wv_d = nc.dram_tensor("wvt", [HIDDEN, OC], bf16, kind="ExternalInput").ap()
    wo_d = nc.dram_tensor("wot", [OC, HIDDEN], bf16, kind="ExternalInput").ap()
    out_d = nc.dram_tensor("partial", [S, HIDDEN], bf16,
                           kind="ExternalOutput").ap()
    mask_d = nc.inline_tensor(_masks_np(), name="masks").ap()
    id_d = nc.inline_tensor(np.eye(128, dtype=BF16), name="ident").ap()

    with tile.TileContext(nc) as tc:
        import contextlib
        with contextlib.ExitStack() as ctx:
            big = ctx.enter_context(tc.tile_pool(name="big", bufs=1))
            tmp = ctx.enter_context(tc.tile_pool(name="tmp", bufs=3))
            pbig = ctx.enter_context(tc.tile_pool(name="pbig", bufs=2,
                                                  space="PSUM"))
            p512 = ctx.enter_context(tc.tile_pool(name="p512", bufs=2,
                                                  space="PSUM"))
            # PSUM tile slots are bank (2KB) granular: pack the small
            # accumulators into two manually-subdivided banks.
            pper = ctx.enter_context(tc.tile_pool(name="pper", bufs=1,
                                                  space="PSUM"))

            # ---- resident tensors ----
            xt_sb = big.tile([128, 8, S], bf16)       # x.T, hidden chunks on dim1
            qt_sb = big.tile([128, S], bf16)          # q.T * 0.125
            kt_sb = big.tile([128, S], bf16)
            v_sb = big.tile([128, NQB, 130], bf16)    # [vA|1|vB|1] per key block
            v0bc = big.tile([128, 130], bf16)         # v[key0] bcast to all parts
            outn_sb = big.tile([128, NQB, 128], bf16)  # attn out, natural [q, hd]
            outt_sb = big.tile([128, NQB, 128], bf16)  # transposed [hd, q]
            p0col_sb = big.tile([128, 64], bf16)       # q0-row probs (col=2qb+h)
            wq_sb = big.tile([128, 8, OC], bf16)
            wk_sb = big.tile([128, 8, OC], bf16)
            wv_sb = big.tile([128, 8, OC], bf16)
            wo_sb = big.tile([128, HIDDEN], bf16)
            mask_sb = big.tile([128, 4, 768], bf16)
            id_sb = big.tile([128, 128], bf16)

            # bankA: q0col [0:64], pso0 [64:194], pstr slots [256:320],[320:384]
            bankA = pper.tile([128, 512], f32, name="bankA")
            # bankB: pso even [0:130], pso odd [192:322], pq0 row [352:482]
            bankB = pper.tile([128, 512], f32, name="bankB")
            q0col = bankA[:, 0:64]      # scoresT(key blocks, q=0), col=2qb+h
            pso0 = bankA[:, 64:194]     # qb0's PV accum (deferred)
            pstr_slots = [bankA[:, 256:320].bitcast(bf16),
                          bankA[:, 320:384].bitcast(bf16)]
            pso_slots = [bankB[:, 0:130], bankB[:, 192:322]]
            pq0 = bankB[0:1, 352:482]   # q0-row PV accum [A 0:65 | B 65:130]

            # ---- constant / weight loads ----
            nc.sync.dma_start(wq_sb, wq_d.rearrange("(c p) o -> p c o", p=128))
            nc.sync.dma_start(wk_sb, wk_d.rearrange("(c p) o -> p c o", p=128))
            nc.sync.dma_start(wv_sb, wv_d.rearrange("(c p) o -> p c o", p=128))
            nc.sync.dma_start(wo_sb, wo_d)
            nc.sync.dma_start(mask_sb, mask_d.rearrange("m p f -> p m f"))
            nc.sync.dma_start(id_sb, id_d)
            nc.vector.memset(v_sb[:, :, 64], 1.0)
            nc.vector.memset(v_sb[:, :, 129], 1.0)
            nc.vector.memset(q0col, 0.0)

            xt_ap = xt_d.rearrange("(c p) s -> p c s", p=128)

            # ---- phase A: projections ----
            for sc in range(8):
                ssl = slice(sc * 512, (sc + 1) * 512)
                nc.sync.dma_start(xt_sb[:, :, ssl], xt_ap[:, :, ssl])

                psv = p512.tile([128, 512], f32, tag="ps512", name="psv")
                psq = p512.tile([128, 512], f32, tag="ps512", name="psq")
                psk = p512.tile([128, 512], f32, tag="ps512", name="psk")
                # interleave v (stationary=xt, N=128) with q/k (N=512) so the
                # frequent v LDWEIGHTS hide under long q/k streams
                for b in range(2):
                    bsl = slice(sc * 512 + b * 128, sc * 512 + b * 128 + 128)
                    for hc in range(8):
                        nc.tensor.matmul(psv[:, b * 128:(b + 1) * 128],
                                         xt_sb[:, hc, bsl], wv_sb[:, hc, :],
                                         start=(hc == 0), stop=(hc == 7))
                for hc in range(8):
                    nc.tensor.matmul(psq, wq_sb[:, hc, :], xt_sb[:, hc, ssl],
                                     start=(hc == 0), stop=(hc == 7))
                for b in range(2, 4):
                    bsl = slice(sc * 512 + b * 128, sc * 512 + b * 128 + 128)
                    for hc in range(8):
                        nc.tensor.matmul(psv[:, b * 128:(b + 1) * 128],
                                         xt_sb[:, hc, bsl], wv_sb[:, hc, :],
                                         start=(hc == 0), stop=(hc == 7))
                for hc in range(8):
                    nc.tensor.matmul(psk, wk_sb[:, hc, :], xt_sb[:, hc, ssl],
                                     start=(hc == 0), stop=(hc == 7))

                # fold the 1/sqrt(hd) = 0.125 softmax scale into q
                nc.vector.tensor_scalar_mul(qt_sb[:, ssl], psq, 0.125)
                nc.scalar.copy(kt_sb[:, ssl], psk)
                # v: [4 blocks][A64|B64] -> v_sb [kb][A|1|B|1] in one copy
                vdst = v_sb[:, sc * 4:sc * 4 + 4, :].rearrange(
                    "p b (h c) -> p b h c", h=2)
                nc.vector.tensor_copy(
                    vdst[:, :, :, 0:64],
                    psv.rearrange("p (b h c) -> p b h c", b=4, h=2))

            # v[key0] broadcast to all partitions (for the rank-1 key0 PV)
            nc.gpsimd.partition_broadcast(v0bc, v_sb[0:1, 0, :])

            # ---- phase B: per query block ----
            for qb in range(NQB):
                qsl = slice(qb * 128, (qb + 1) * 128)
                kbs, whead, mslot, strip = _qb_plan(qb)
                nkb = len(kbs)
                W = (strip + 128) if strip is not None else 2 * whead

                pss = pbig.tile([128, 896], f32, tag="pss", name="pss")
                for h in range(2):
                    bp = 64 * h
                    for j, kb in enumerate(kbs):
                        off = h * whead + j * 128
                        nc.tensor.matmul(
                            pss[:, off:off + 128],
                            kt_sb[bp:bp + 64, kb * 128:(kb + 1) * 128],
                            qt_sb[bp:bp + 64, qsl],
                            start=True, stop=True)
                        if kb == qb and qb >= 2:
                            # rider: q=0 scores vs this key block (reuses the
                            # diag stationary position in the PE queue)
                            nc.tensor.matmul(
                                q0col[:, 2 * qb + h:2 * qb + h + 1],
                                kt_sb[bp:bp + 64, kb * 128:(kb + 1) * 128],
                                qt_sb[bp:bp + 64, 0:1],
                                start=True, stop=True)
                    if strip is not None:
                        # key-0 scores for this qb's queries: [1, 128] row at
                        # partition 0 (A) / 32 (B)
                        sp = 32 * h
                        nc.tensor.matmul(
                            pss[sp:sp + 1, strip:strip + 128],
                            kt_sb[bp:bp + 64, 0:1],
                            qt_sb[bp:bp + 64, qsl],
                            start=True, stop=True)

                probs = tmp.tile([128, 896], bf16, tag="probs", name="probs")
                nc.scalar.activation(probs[:, :W], pss[:, :W], Exp)
                nc.gpsimd.tensor_mul(probs[:, :2 * whead],
                                     probs[:, :2 * whead],
                                     mask_sb[:, mslot, :2 * whead])

                pso_t = pso0 if qb == 0 else pso_slots[qb % 2]
                for h in range(2):
                    hs = slice(65 * h, 65 * h + 65)
                    for j, kb in enumerate(kbs):
                        off = h * whead + j * 128
                        nc.tensor.matmul(
                            pso_t[:, hs], probs[:, off:off + 128],
                            v_sb[:, kb, hs],
                            start=(j == 0), stop=(strip is None and j == nkb - 1),
                            skip_group_check=True)
                    if strip is not None:
                        sp = 32 * h
                        nc.tensor.matmul(
                            pso_t[:, hs],
                            probs[sp:sp + 1, strip:strip + 128],
                            v0bc[sp:sp + 1, hs],
                            start=False, stop=True, skip_group_check=True)

                if qb == 0:
                    continue  # normalize/out-proj deferred to the end

                _normalize_project(nc, tmp, p512, pstr_slots[qb % 2], pso_t,
                                   outn_sb, outt_sb, id_sb, wo_sb, out_d, qb,
                                   f32, bf16)

            # ---- tail: q0 row (keys 256+) and deferred qb 0 ----
            nc.scalar.activation(p0col_sb, q0col, Exp)
            for h in range(2):
                hs = slice(65 * h, 65 * h + 65)
                for kb in range(2, NQB):
                    nc.tensor.matmul(
                        pq0[:, hs], p0col_sb[:, 2 * kb + h:2 * kb + h + 1],
                        v_sb[:, kb, hs],
                        start=(kb == 2), stop=(kb == NQB - 1),
                        skip_group_check=True)
            # patch q0 row (numerator + denominator) into qb0's accumulator
            pq0_sb = tmp.tile([1, 130], f32, tag="pq0sb", name="pq0_sb")
            nc.vector.tensor_copy(pq0_sb, pq0)
            nc.vector.tensor_add(pso0[0:1, :], pso0[0:1, :], pq0_sb)
            _normalize_project(nc, tmp, p512, pstr_slots[0], pso0, outn_sb,
                               outt_sb, id_sb, wo_sb, out_d, 0, f32, bf16)

    nc.compile()
    return nc


def _normalize_project(nc, tmp, p512, pstr, pso_t, outn_sb, outt_sb, id_sb,
                       wo_sb, out_d, qb, f32, bf16):
    """pso [q, A64|dA|B64|dB] -> outn=num/denom -> outt (hd, q) -> @Wo -> DMA"""
    pso_h = pso_t.rearrange("p (h c) -> p h c", h=2)
    recip = tmp.tile([128, 2], f32, tag="recip", name="recip")
    nc.vector.reciprocal(recip, pso_h[:, :, 64])
    for h in range(2):
        nc.vector.tensor_scalar_mul(outn_sb[:, qb, 64 * h:64 * h + 64],
                                    pso_h[:, h, 0:64], recip[:, h:h + 1])
    nc.tensor.transpose(pstr, outn_sb[:, qb, :], id_sb)
    nc.vector.tensor_copy(outt_sb[:, qb, :], pstr)
    stage = tmp.tile([128, HIDDEN], bf16, tag="stage", name="stage")
    for oc in range(2):
        psp = p512.tile([128, 512], f32, tag="ps512", name="psp")
        nc.tensor.matmul(psp, outt_sb[:, qb, :],
                         wo_sb[:, oc * 512:(oc + 1) * 512],
                         start=True, stop=True)
        if oc == 0:
            nc.vector.tensor_copy(stage[:, oc * 512:(oc + 1) * 512], psp)
        else:
            nc.scalar.copy(stage[:, oc * 512:(oc + 1) * 512], psp)
    nc.sync.dma_start(out_d[qb * 128:(qb + 1) * 128, :], stage)


def kernel(x, Wq, Wk, Wv, Wo):
    from concourse import bass_utils

    x = np.asarray(x)
    B = x.shape[0]
    xt = np.ascontiguousarray(np.asarray(x)[0].T.astype(BF16))
    in_maps = []
    for d in range(N_CORES):
        rs = slice(OC * d, OC * (d + 1))
        in_maps.append({
            "xt": xt,
            "wqt": np.ascontiguousarray(np.asarray(Wq)[rs, :].T.astype(BF16)),
            "wkt": np.ascontiguousarray(np.asarray(Wk)[rs, :].T.astype(BF16)),
            "wvt": np.ascontiguousarray(np.asarray(Wv)[rs, :].T.astype(BF16)),
            "wot": np.ascontiguousarray(np.asarray(Wo)[:, rs].T.astype(BF16)),
        })

    if "nc" not in _CACHE:
        _CACHE["nc"] = _build()
    nc = _CACHE["nc"]

    res = bass_utils.run_bass_kernel_spmd(
        nc, in_maps, core_ids=list(range(N_CORES)),
        trace=bool(os.environ.get("KERNEL_TRACE")))
    global LAST_RESULTS
    LAST_RESULTS = res

    out = np.zeros((S, HIDDEN), np.float64)
    for r in res.results:
        out += r["partial"].astype(np.float64)
    return out.reshape(B, S, HIDDEN).astype(np.float32)
